# revision 28
# baseline (speedup 1.0000x reference)
"""Distributed attention kernel for trn2 (8 NeuronCores).

Problem: B=16, S=4096, D=64 attention, out = softmax(Q K^T / sqrt(D)) V.
Sharding: batch dim B across 8 cores (2 batches per core), no collectives.

Per-core dataflow (everything in "transposed score" layout; PE assumed
pinned at 1.2 GHz, so all matmuls are packed with tile_position
concurrency):
  - K^T loaded via bf16 DRAM scratch + x-bar DMA transpose in an
    even/odd-s interleaved layout: ktp[128, 2048], top half = K^T of
    even s, bottom half = odd s.
  - Q^T duplicated onto both partition halves (qt2[128, 4096]) via
    doubled load + TensorE transposes, so 2x2-tiled score matmuls can
    source rhs from either half.
  - V loaded even/odd interleaved (vq[128, g, parity, 64]).
  - Per 512-wide q tile, per group g (256 consecutive k):
      S-quad: 4 concurrent K=64/M=64 matmuls -> S^T for even k (bank 0)
              and odd k (bank 1) of sp[128, 1024].
      exp:    one ScalarE activation [128, 1024] psum -> et bf16.
      AV-quad: 4 concurrent M=32 col-tiled matmuls accumulate
              O_even (ot2[0:64]) and O_odd (ot2[64:128]).
      sums:   every 2nd group, 4 concurrent M=1 matmuls with a ones
              vector accumulate sum(exp) into rows {0,32,64,96} of rs.
  - Phase C: copy to SBUF, accumulating PE transposes add the even/odd
    halves, a tiny matmul with a 4-hot selector vector folds the 4 sum
    rows into r[q], reciprocal + per-partition scale, contiguous DMA.
"""

import numpy as np

import concourse.bass as bass
import concourse.mybir as mybir
from concourse import bacc
from concourse.tile import TileContext
from concourse.bass_utils import run_bass_kernel_spmd
from concourse.masks import make_identity

B, S, D = 16, 4096, 64
N_CORES = 8
BS = B // N_CORES  # batches per core
SCALE = 1.0 / np.sqrt(D)  # 0.125
F32 = mybir.dt.float32
BF16 = mybir.dt.bfloat16

QTW = 512  # q-tile width
NG = S // 256  # 16 groups of 256 k (even/odd chunk pair) per q tile
N_QT = S // QTW  # 8 q tiles
NCH = S // 128  # 32 chunks of 128 rows


def emit_phase_a_alloc(nc, pools):
    # Layouts (k-permutation invariant):
    #   ktp[0:64, a]   = K[a, :]^T        for a in [0, 2048)   (k half 0)
    #   ktp[64:128, a] = K[2048 + a, :]^T                      (k half 1)
    #   qt2 = Q^T duplicated on both partition halves
    #   vq4[:, g, h, d]: row r = V[h * 2048 + g * 128 + r, d]
    qkp, vp = pools["qk"], pools["v"]
    qt2 = qkp.tile([128, S], BF16, tag="qt2")
    ktp = qkp.tile([128, 2048], BF16, tag="ktp")
    vq = vp.tile([128, NG * 2 * 64], BF16, tag="vq")
    vq4 = vq[:].rearrange("p (g h d) -> p g h d", h=2, d=64)
    return qt2, ktp, vq4


def emit_phase_a_k(nc, pools, Kd, b, identB, handles, piece, bg=None):
    natp, tpp = pools["nat"], pools["small"]
    qt2, ktp, vq4 = handles
    c0 = piece * 8
    knat2 = natp.tile([128, 8 * 2 * 64], BF16, tag=f"knat{piece % 2}")
    kn4 = knat2[:].rearrange("p (c two d) -> p c two d", two=2, d=64)
    srcK = Kd[b, c0 * 128 : (c0 + 8) * 128].rearrange("(c p) d -> p c d", p=128)
    nc.gpsimd.dma_start(out=kn4[:, :, 0, :], in_=srcK)
    nc.vector.tensor_copy(kn4[:, :, 1, :], kn4[:, :, 0, :])
    def work(i, trg):
        c = c0 + i
        tr = trg[:, i * 128 : (i + 1) * 128]
        nc.tensor.transpose(tr, knat2[:, i * 128 : (i + 1) * 128], identB[:])
        if c < 16:
            nc.vector.tensor_copy(
                ktp[0:64, c * 128 : (c + 1) * 128], tr[0:64, :]
            )
        else:
            cc = c - 16
            nc.vector.tensor_copy(
                ktp[64:128, cc * 128 : (cc + 1) * 128], tr[64:128, :]
            )

    if bg is None:
        trg = tpp.tile([128, 8 * 128], BF16, tag="small")
        for i in range(8):
            work(i, trg)
    else:
        state = {}

        def closure(i):
            if "trg" not in state:
                state["trg"] = tpp.tile([128, 8 * 128], BF16, tag="small", name="trg")
            work(i, state["trg"])

        for i in range(8):
            bg.append(lambda i=i: closure(i))


def emit_phase_a_q(nc, pools, Qd, b, identB, handles, piece, bg=None):
    natp, tpp = pools["nat"], pools["small"]
    qt2, ktp, vq4 = handles
    c0 = piece * 8
    qnat2 = natp.tile([128, 8 * 2 * 64], BF16, tag=f"qnat{piece % 2}")
    qn4 = qnat2[:].rearrange("p (c two d) -> p c two d", two=2, d=64)
    srcQ = Qd[b, c0 * 128 : (c0 + 8) * 128].rearrange("(c p) d -> p c d", p=128)
    nc.gpsimd.dma_start(out=qn4[:, :, 0, :], in_=srcQ)
    nc.vector.tensor_copy(qn4[:, :, 1, :], qn4[:, :, 0, :])
    def work(i, trg):
        c = c0 + i
        tr = trg[:, i * 128 : (i + 1) * 128]
        nc.tensor.transpose(tr, qnat2[:, i * 128 : (i + 1) * 128], identB[:])
        nc.vector.tensor_copy(qt2[:, c * 128 : (c + 1) * 128], tr[:])

    if bg is None:
        trg = tpp.tile([128, 8 * 128], BF16, tag="small")
        for i in range(8):
            work(i, trg)
    else:
        state = {}

        def closure(i):
            if "trg" not in state:
                state["trg"] = tpp.tile([128, 8 * 128], BF16, tag="small", name="trg")
            work(i, state["trg"])

        for i in range(8):
            bg.append(lambda i=i: closure(i))


def emit_phase_a_v(nc, pools, Vd, b, handles):
    qt2, ktp, vq4 = handles
    for h in range(2):
        nc.gpsimd.dma_start(
            out=vq4[:, :, h, :],
            in_=Vd[b, h * 2048 : (h + 1) * 2048].rearrange(
                "(g r) d -> r g d", r=128
            ),
        )


def emit_phase_b(nc, pools, Od, b, qt2, ktp, vq4, ones, wsel, ident2, after_qt0, bg):
    spp, opp, rsp, tpp, ep, fp = (
        pools["sp"], pools["ot"], pools["rs"], pools["small"],
        pools["et"], pools["fin"],
    )
    NGG = N_QT * NG  # 128 groups per batch
    sp_tiles = {}

    def emit_squad(gg):
        qi, g = gg // NG, gg % NG
        qc_lo = qt2[0:64, qi * QTW : (qi + 1) * QTW]
        qc_hi = qt2[64:128, qi * QTW : (qi + 1) * QTW]
        sp = spp.tile([128, QTW * 2], F32, tag="sp")
        sp_tiles[gg] = sp
        nc.tensor.matmul(
            sp[0:64, 0:QTW], ktp[0:64, g * 128 : g * 128 + 64], qc_lo,
            start=True, stop=True, skip_group_check=True,
        )
        nc.tensor.matmul(
            sp[64:128, 0:QTW], ktp[0:64, g * 128 + 64 : g * 128 + 128], qc_lo,
            start=True, stop=True, skip_group_check=True,
        )
        nc.tensor.matmul(
            sp[0:64, QTW : 2 * QTW], ktp[64:128, g * 128 : g * 128 + 64], qc_hi,
            start=True, stop=True, skip_group_check=True,
        )
        nc.tensor.matmul(
            sp[64:128, QTW : 2 * QTW],
            ktp[64:128, g * 128 + 64 : g * 128 + 128], qc_hi,
            start=True, stop=True, skip_group_check=True,
        )

    emit_squad(0)
    emit_squad(1)
    ot2 = rs = None
    et_prev = None
    for gg in range(NGG):
        qi, g = gg // NG, gg % NG
        if g == 0:
            ot2 = opp.tile([128, QTW], F32, tag="ot2")
            rs = rsp.tile([97, QTW], F32, tag="rs")
        sp = sp_tiles.pop(gg)
        et = ep.tile([128, QTW * 2], BF16, tag="et")
        nc.scalar.activation(
            et[:], sp[:], mybir.ActivationFunctionType.Exp, scale=SCALE
        )
        if gg + 2 < NGG:
            emit_squad(gg + 2)
        for t in range(4):
            par = t // 2
            nc.tensor.matmul(
                ot2[32 * t : 32 * (t + 1), :],
                vq4[:, g, par, 32 * (t % 2) : 32 * (t % 2 + 1)],
                et[:, par * QTW : (par + 1) * QTW],
                start=(g == 0), stop=(g == NG - 1), skip_group_check=True,
                tile_position=(0, 32 * t),
            )
        if g % 2 == 1:
            for t, (esrc, half) in enumerate(
                [(et_prev, 0), (et_prev, 1), (et, 0), (et, 1)]
            ):
                nc.tensor.matmul(
                    rs[32 * t : 32 * t + 1, :],
                    ones[:],
                    esrc[:, half * QTW : (half + 1) * QTW],
                    start=(g == 1), stop=(g == NG - 1),
                    skip_group_check=True, tile_position=(0, 32 * t),
                )
        et_prev = et
        for _ in range(3):
            if bg:
                bg.pop(0)()

        if g == NG - 1:
            # ---- Phase C for q-tile qi ----
            osb = fp.tile([128, QTW], BF16, tag="osb")
            nc.vector.tensor_copy(osb[:], ot2[:])
            rsb = fp.tile([97, QTW], BF16, tag="rsb")
            nc.vector.tensor_copy(rsb[:], rs[:])
            ctp = tpp.tile([128, 4 * 64], F32, tag="small")
            rcol = rsp.tile([128, 4], F32, tag="rs")
            for j in range(QTW // 128):
                js = slice(j * 128, (j + 1) * 128)
                nc.tensor.matmul(
                    ctp[:, j * 64 : (j + 1) * 64], osb[:, js], ident2[:],
                    start=True, stop=True, skip_group_check=True,
                )
                nc.tensor.matmul(
                    rcol[:, j : j + 1], rsb[:, js], wsel[:],
                    start=True, stop=True, skip_group_check=True,
                )
            rinv = fp.tile([128, 4], F32, tag="rinv")
            nc.vector.reciprocal(rinv[:], rcol[:])
            ob = fp.tile([128, 4 * 64], F32, tag="ob")
            for j in range(QTW // 128):
                nc.vector.tensor_scalar_mul(
                    ob[:, j * 64 : (j + 1) * 64],
                    ctp[:, j * 64 : (j + 1) * 64], rinv[:, j : j + 1]
                )
            nc.sync.dma_start(
                out=Od[b, qi * QTW : (qi + 1) * QTW].rearrange(
                    "(j p) d -> p j d", p=128
                ),
                in_=ob[:].rearrange("p (j d) -> p j d", d=64),
            )
            if after_qt0 is not None:
                after_qt0(qi)


def build_body(nc, tc, Qd, Kd, Vd, Od):
    with (
        tc.tile_pool(name="const", bufs=1) as constp,
        tc.tile_pool(name="qk", bufs=2) as qkp,
        tc.tile_pool(name="v", bufs=2) as vp,
        tc.tile_pool(name="nat", bufs=2) as natp,
        tc.tile_pool(name="sp", bufs=2, space="PSUM") as spp,
        tc.tile_pool(name="ot", bufs=1, space="PSUM") as opp,
        tc.tile_pool(name="rs", bufs=1, space="PSUM") as rsp,
        tc.tile_pool(name="small", bufs=2, space="PSUM") as tpp,
        tc.tile_pool(name="et", bufs=6) as ep,
        tc.tile_pool(name="fin", bufs=4) as fp,
    ):
        pools = {
            "qk": qkp, "v": vp, "nat": natp, "sp": spp,
            "ot": opp, "rs": rsp, "small": tpp, "et": ep, "fin": fp,
        }
        ident2 = constp.tile([128, 64], BF16)
        nc.gpsimd.memset(ident2[:], 0.0)
        for half in range(2):
            nc.gpsimd.affine_select(
                out=ident2[64 * half : 64 * (half + 1), :],
                in_=ident2[64 * half : 64 * (half + 1), :],
                compare_op=mybir.AluOpType.not_equal, fill=1.0, base=0,
                pattern=[[-1, 64]], channel_multiplier=1,
            )
        identB = constp.tile([128, 128], BF16)
        make_identity(nc, identB[:])
        ones = constp.tile([128, 1], BF16)
        nc.gpsimd.memset(ones[:], 1.0)
        wsel = constp.tile([97, 1], BF16)
        nc.gpsimd.memset(wsel[:], 0.0)
        for t in range(4):
            nc.gpsimd.memset(wsel[32 * t : 32 * t + 1, :], 1.0)

        handles = [None] * BS
        bg = []
        handles[0] = emit_phase_a_alloc(nc, pools)
        for piece in (0, 2):
            emit_phase_a_k(nc, pools, Kd, 0, identB, handles[0], piece)
        emit_phase_a_q(nc, pools, Qd, 0, identB, handles[0], 0)
        emit_phase_a_v(nc, pools, Vd, 0, handles[0])
        for piece in (1, 3):
            emit_phase_a_k(nc, pools, Kd, 0, identB, handles[0], piece, bg)
        for piece in (1, 2, 3):
            emit_phase_a_q(nc, pools, Qd, 0, identB, handles[0], piece, bg)
        for b in range(BS):

            def prefetch(qi, b=b, bg=bg):
                if b + 1 >= BS:
                    return
                if qi == 0:
                    handles[b + 1] = emit_phase_a_alloc(nc, pools)
                    emit_phase_a_k(nc, pools, Kd, b + 1, identB, handles[b + 1], 0, bg)
                elif qi == 1:
                    emit_phase_a_k(nc, pools, Kd, b + 1, identB, handles[b + 1], 2, bg)
                    emit_phase_a_q(nc, pools, Qd, b + 1, identB, handles[b + 1], 0, bg)
                elif qi == 2:
                    emit_phase_a_v(nc, pools, Vd, b + 1, handles[b + 1])
                    emit_phase_a_k(nc, pools, Kd, b + 1, identB, handles[b + 1], 1, bg)
                elif qi == 3:
                    emit_phase_a_k(nc, pools, Kd, b + 1, identB, handles[b + 1], 3, bg)
                    emit_phase_a_q(nc, pools, Qd, b + 1, identB, handles[b + 1], 1, bg)
                elif qi == 4:
                    emit_phase_a_q(nc, pools, Qd, b + 1, identB, handles[b + 1], 2, bg)
                elif qi == 5:
                    emit_phase_a_q(nc, pools, Qd, b + 1, identB, handles[b + 1], 3, bg)

            qt2, ktp, vq4 = handles[b]
            emit_phase_b(
                nc, pools, Od, b, qt2, ktp, vq4, ones, wsel, ident2, prefetch, bg
            )
            while bg:
                bg.pop(0)()


_nc_cache = None


def build_nc():
    global _nc_cache
    if _nc_cache is not None:
        return _nc_cache
    nc = bacc.Bacc(None, target_bir_lowering=False)
    Qd = nc.declare_dram_parameter("Q", [BS, S, D], F32, isOutput=False)
    Kd = nc.declare_dram_parameter("K", [BS, S, D], F32, isOutput=False)
    Vd = nc.declare_dram_parameter("V", [BS, S, D], F32, isOutput=False)
    Od = nc.declare_dram_parameter("out", [BS, S, D], F32, isOutput=True)
    with TileContext(nc) as tc:
        build_body(nc, tc, Qd, Kd, Vd, Od)
    nc.finalize()
    _nc_cache = nc
    return nc


def kernel(Q, K, V):
    Q = np.asarray(Q, dtype=np.float32)
    K = np.asarray(K, dtype=np.float32)
    V = np.asarray(V, dtype=np.float32)
    nc = build_nc()
    in_maps = [
        {
            "Q": np.ascontiguousarray(Q[i * BS : (i + 1) * BS]),
            "K": np.ascontiguousarray(K[i * BS : (i + 1) * BS]),
            "V": np.ascontiguousarray(V[i * BS : (i + 1) * BS]),
        }
        for i in range(N_CORES)
    ]
    res = run_bass_kernel_spmd(nc, in_maps, core_ids=list(range(N_CORES)))
    return np.concatenate([res.results[i]["out"] for i in range(N_CORES)], axis=0)


# revision 29
# speedup vs baseline: 1.0739x; 1.0739x over previous
"""Distributed attention kernel for trn2 (8 NeuronCores).

Problem: B=16, S=4096, D=64 attention, out = softmax(Q K^T / sqrt(D)) V.
Sharding: batch dim B across 8 cores (2 batches per core), no collectives.

Per-core dataflow (everything in "transposed score" layout; PE assumed
pinned at 1.2 GHz, so all matmuls are packed with tile_position
concurrency):
  - K^T loaded via bf16 DRAM scratch + x-bar DMA transpose in an
    even/odd-s interleaved layout: ktp[128, 2048], top half = K^T of
    even s, bottom half = odd s.
  - Q^T duplicated onto both partition halves (qt2[128, 4096]) via
    doubled load + TensorE transposes, so 2x2-tiled score matmuls can
    source rhs from either half.
  - V loaded even/odd interleaved (vq[128, g, parity, 64]).
  - Per 512-wide q tile, per group g (256 consecutive k):
      S-quad: 4 concurrent K=64/M=64 matmuls -> S^T for even k (bank 0)
              and odd k (bank 1) of sp[128, 1024].
      exp:    one ScalarE activation [128, 1024] psum -> et bf16.
      AV-quad: 4 concurrent M=32 col-tiled matmuls accumulate
              O_even (ot2[0:64]) and O_odd (ot2[64:128]).
      sums:   every 2nd group, 4 concurrent M=1 matmuls with a ones
              vector accumulate sum(exp) into rows {0,32,64,96} of rs.
  - Phase C: copy to SBUF, accumulating PE transposes add the even/odd
    halves, a tiny matmul with a 4-hot selector vector folds the 4 sum
    rows into r[q], reciprocal + per-partition scale, contiguous DMA.
"""

import numpy as np

import concourse.bass as bass
import concourse.mybir as mybir
from concourse import bacc
from concourse.tile import TileContext
from concourse.bass_utils import run_bass_kernel_spmd
from concourse.masks import make_identity

B, S, D = 16, 4096, 64
N_CORES = 8
BS = B // N_CORES  # batches per core
SCALE = 1.0 / np.sqrt(D)  # 0.125
F32 = mybir.dt.float32
BF16 = mybir.dt.bfloat16

QTW = 512  # q-tile width
NG = S // 256  # 16 groups of 256 k (even/odd chunk pair) per q tile
N_QT = S // QTW  # 8 q tiles
NCH = S // 128  # 32 chunks of 128 rows


def emit_phase_a_alloc(nc, pools):
    # Layouts (k-permutation invariant):
    #   ktp[0:64, a]   = K[a, :]^T        for a in [0, 2048)   (k half 0)
    #   ktp[64:128, a] = K[2048 + a, :]^T                      (k half 1)
    #   qt2 = Q^T duplicated on both partition halves
    #   vq4[:, g, h, d]: row r = V[h * 2048 + g * 128 + r, d]
    qkp, vp = pools["qk"], pools["v"]
    qt2 = qkp.tile([128, S], BF16, tag="qt2")
    ktp = qkp.tile([128, 2048], BF16, tag="ktp")
    vq = vp.tile([128, NG * 2 * 64], BF16, tag="vq")
    vq4 = vq[:].rearrange("p (g h d) -> p g h d", h=2, d=64)
    return qt2, ktp, vq4


def emit_phase_a_k(nc, pools, Kd, b, identB, handles, piece, bg=None):
    natp, tpp = pools["nat"], pools["small"]
    qt2, ktp, vq4 = handles
    p0 = piece * 4  # 4 pairs per piece; pair pr = chunks (pr, 16+pr)
    knp = natp.tile([128, 4 * 2 * 64], BF16, tag=f"knat{piece % 2}", name="knp")
    kn4 = knp[:].rearrange("p (pr h d) -> p pr h d", h=2, d=64)
    for h in range(2):
        c0 = h * 16 + p0
        nc.gpsimd.dma_start(
            out=kn4[:, :, h, :],
            in_=Kd[b, c0 * 128 : (c0 + 4) * 128].rearrange(
                "(pr r) d -> r pr d", r=128
            ),
        )

    def work(i, trg, off=0):
        pr = p0 + i
        tr = trg[:, off * 128 : (off + 1) * 128]
        nc.tensor.transpose(tr, knp[:, i * 128 : (i + 1) * 128], identB[:])
        nc.vector.tensor_copy(ktp[:, pr * 128 : (pr + 1) * 128], tr[:])

    if bg is None:
        trg = tpp.tile([128, 4 * 128], BF16, tag="small", name="trgk")
        for i in range(4):
            work(i, trg, i)
    else:
        def closure(i):
            trg1 = tpp.tile([128, 128], BF16, tag="small", name="trg1")
            work(i, trg1, 0)

        for i in range(4):
            bg.append(lambda i=i: closure(i))


def emit_phase_a_q(nc, pools, Qd, b, identB, handles, piece, bg=None):
    natp, tpp = pools["nat"], pools["small"]
    qt2, ktp, vq4 = handles
    c0 = piece * 8
    qnat2 = natp.tile([128, 8 * 2 * 64], BF16, tag=f"qnat{piece % 2}")
    qn4 = qnat2[:].rearrange("p (c two d) -> p c two d", two=2, d=64)
    srcQ = Qd[b, c0 * 128 : (c0 + 8) * 128].rearrange("(c p) d -> p c d", p=128)
    nc.gpsimd.dma_start(out=qn4[:, :, 0, :], in_=srcQ)
    nc.gpsimd.dma_start(out=qn4[:, :, 1, :], in_=srcQ)
    def work(i, trg):
        c = c0 + i
        tr = trg[:, i * 128 : (i + 1) * 128]
        nc.tensor.transpose(tr, qnat2[:, i * 128 : (i + 1) * 128], identB[:])
        nc.vector.tensor_copy(qt2[:, c * 128 : (c + 1) * 128], tr[:])

    if bg is None:
        trg = tpp.tile([128, 8 * 128], BF16, tag="small")
        for i in range(8):
            work(i, trg)
    else:
        state = {}

        def closure(i):
            if "trg" not in state:
                state["trg"] = tpp.tile([128, 8 * 128], BF16, tag="small", name="trg")
            work(i, state["trg"])

        for i in range(8):
            bg.append(lambda i=i: closure(i))


def emit_phase_a_v(nc, pools, Vd, b, handles):
    qt2, ktp, vq4 = handles
    for h in range(2):
        nc.gpsimd.dma_start(
            out=vq4[:, :, h, :],
            in_=Vd[b, h * 2048 : (h + 1) * 2048].rearrange(
                "(g r) d -> r g d", r=128
            ),
        )


def emit_phase_b(nc, pools, Od, b, qt2, ktp, vq4, ones, wsel, ident2, after_qt0, bg):
    spp, opp, rsp, tpp, ep, fp = (
        pools["sp"], pools["ot"], pools["rs"], pools["small"],
        pools["et"], pools["fin"],
    )
    NGG = N_QT * NG  # 128 groups per batch
    sp_tiles = {}

    def emit_squad(gg):
        qi, g = gg // NG, gg % NG
        qc_lo = qt2[0:64, qi * QTW : (qi + 1) * QTW]
        qc_hi = qt2[64:128, qi * QTW : (qi + 1) * QTW]
        sp = spp.tile([128, QTW * 2], F32, tag="sp")
        sp_tiles[gg] = sp
        nc.tensor.matmul(
            sp[0:64, 0:QTW], ktp[0:64, g * 128 : g * 128 + 64], qc_lo,
            start=True, stop=True, skip_group_check=True,
        )
        nc.tensor.matmul(
            sp[64:128, 0:QTW], ktp[0:64, g * 128 + 64 : g * 128 + 128], qc_lo,
            start=True, stop=True, skip_group_check=True,
        )
        nc.tensor.matmul(
            sp[0:64, QTW : 2 * QTW], ktp[64:128, g * 128 : g * 128 + 64], qc_hi,
            start=True, stop=True, skip_group_check=True,
        )
        nc.tensor.matmul(
            sp[64:128, QTW : 2 * QTW],
            ktp[64:128, g * 128 + 64 : g * 128 + 128], qc_hi,
            start=True, stop=True, skip_group_check=True,
        )

    emit_squad(0)
    emit_squad(1)
    ot2 = rs = None
    et_prev = None
    for gg in range(NGG):
        qi, g = gg // NG, gg % NG
        if g == 0:
            ot2 = opp.tile([128, QTW], F32, tag="ot2")
            rs = rsp.tile([97, QTW], F32, tag="rs")
        sp = sp_tiles.pop(gg)
        et = ep.tile([128, QTW * 2], BF16, tag="et")
        nc.scalar.activation(
            et[:], sp[:], mybir.ActivationFunctionType.Exp, scale=SCALE
        )
        if gg + 2 < NGG:
            emit_squad(gg + 2)
        for t in range(4):
            par = t // 2
            nc.tensor.matmul(
                ot2[32 * t : 32 * (t + 1), :],
                vq4[:, g, par, 32 * (t % 2) : 32 * (t % 2 + 1)],
                et[:, par * QTW : (par + 1) * QTW],
                start=(g == 0), stop=(g == NG - 1), skip_group_check=True,
                tile_position=(0, 32 * t),
            )
        if g % 2 == 1:
            for t, (esrc, half) in enumerate(
                [(et_prev, 0), (et_prev, 1), (et, 0), (et, 1)]
            ):
                nc.tensor.matmul(
                    rs[32 * t : 32 * t + 1, :],
                    ones[:],
                    esrc[:, half * QTW : (half + 1) * QTW],
                    start=(g == 1), stop=(g == NG - 1),
                    skip_group_check=True, tile_position=(0, 32 * t),
                )
        et_prev = et
        for _ in range(3):
            if bg:
                bg.pop(0)()

        if g == NG - 1:
            # ---- Phase C for q-tile qi ----
            osb = fp.tile([128, QTW], BF16, tag="osb")
            nc.vector.tensor_copy(osb[:], ot2[:])
            rsb = fp.tile([97, QTW], BF16, tag="rsb")
            nc.vector.tensor_copy(rsb[:], rs[:])
            ctp = tpp.tile([128, 4 * 64], F32, tag="small")
            rcol = rsp.tile([128, 4], F32, tag="rs")
            for j in range(QTW // 128):
                js = slice(j * 128, (j + 1) * 128)
                nc.tensor.matmul(
                    ctp[:, j * 64 : (j + 1) * 64], osb[:, js], ident2[:],
                    start=True, stop=True, skip_group_check=True,
                )
                nc.tensor.matmul(
                    rcol[:, j : j + 1], rsb[:, js], wsel[:],
                    start=True, stop=True, skip_group_check=True,
                )
            rinv = fp.tile([128, 4], F32, tag="rinv")
            nc.vector.reciprocal(rinv[:], rcol[:])
            ob = fp.tile([128, 4 * 64], F32, tag="ob")
            for j in range(QTW // 128):
                nc.vector.tensor_scalar_mul(
                    ob[:, j * 64 : (j + 1) * 64],
                    ctp[:, j * 64 : (j + 1) * 64], rinv[:, j : j + 1]
                )
            nc.sync.dma_start(
                out=Od[b, qi * QTW : (qi + 1) * QTW].rearrange(
                    "(j p) d -> p j d", p=128
                ),
                in_=ob[:].rearrange("p (j d) -> p j d", d=64),
            )
            if after_qt0 is not None:
                after_qt0(qi)


def build_body(nc, tc, Qd, Kd, Vd, Od):
    with (
        tc.tile_pool(name="const", bufs=1) as constp,
        tc.tile_pool(name="qk", bufs=2) as qkp,
        tc.tile_pool(name="v", bufs=2) as vp,
        tc.tile_pool(name="nat", bufs=2) as natp,
        tc.tile_pool(name="sp", bufs=2, space="PSUM") as spp,
        tc.tile_pool(name="ot", bufs=1, space="PSUM") as opp,
        tc.tile_pool(name="rs", bufs=1, space="PSUM") as rsp,
        tc.tile_pool(name="small", bufs=2, space="PSUM") as tpp,
        tc.tile_pool(name="et", bufs=6) as ep,
        tc.tile_pool(name="fin", bufs=4) as fp,
    ):
        pools = {
            "qk": qkp, "v": vp, "nat": natp, "sp": spp,
            "ot": opp, "rs": rsp, "small": tpp, "et": ep, "fin": fp,
        }
        ident2 = constp.tile([128, 64], BF16)
        nc.gpsimd.memset(ident2[:], 0.0)
        for half in range(2):
            nc.gpsimd.affine_select(
                out=ident2[64 * half : 64 * (half + 1), :],
                in_=ident2[64 * half : 64 * (half + 1), :],
                compare_op=mybir.AluOpType.not_equal, fill=1.0, base=0,
                pattern=[[-1, 64]], channel_multiplier=1,
            )
        identB = constp.tile([128, 128], BF16)
        make_identity(nc, identB[:])
        ones = constp.tile([128, 1], BF16)
        nc.gpsimd.memset(ones[:], 1.0)
        wsel = constp.tile([97, 1], BF16)
        nc.gpsimd.memset(wsel[:], 0.0)
        for t in range(4):
            nc.gpsimd.memset(wsel[32 * t : 32 * t + 1, :], 1.0)

        handles = [None] * BS
        bg = []
        handles[0] = emit_phase_a_alloc(nc, pools)
        for piece in (0, 2):
            emit_phase_a_k(nc, pools, Kd, 0, identB, handles[0], piece)
        emit_phase_a_q(nc, pools, Qd, 0, identB, handles[0], 0)
        emit_phase_a_v(nc, pools, Vd, 0, handles[0])
        for piece in (1, 3):
            emit_phase_a_k(nc, pools, Kd, 0, identB, handles[0], piece, bg)
        for piece in (1, 2, 3):
            emit_phase_a_q(nc, pools, Qd, 0, identB, handles[0], piece, bg)
        for b in range(BS):

            def prefetch(qi, b=b, bg=bg):
                if b + 1 >= BS:
                    return
                if qi == 0:
                    handles[b + 1] = emit_phase_a_alloc(nc, pools)
                    emit_phase_a_k(nc, pools, Kd, b + 1, identB, handles[b + 1], 0, bg)
                elif qi == 1:
                    emit_phase_a_k(nc, pools, Kd, b + 1, identB, handles[b + 1], 2, bg)
                    emit_phase_a_q(nc, pools, Qd, b + 1, identB, handles[b + 1], 0, bg)
                elif qi == 2:
                    emit_phase_a_v(nc, pools, Vd, b + 1, handles[b + 1])
                    emit_phase_a_k(nc, pools, Kd, b + 1, identB, handles[b + 1], 1, bg)
                elif qi == 3:
                    emit_phase_a_k(nc, pools, Kd, b + 1, identB, handles[b + 1], 3, bg)
                    emit_phase_a_q(nc, pools, Qd, b + 1, identB, handles[b + 1], 1, bg)
                elif qi == 4:
                    emit_phase_a_q(nc, pools, Qd, b + 1, identB, handles[b + 1], 2, bg)
                elif qi == 5:
                    emit_phase_a_q(nc, pools, Qd, b + 1, identB, handles[b + 1], 3, bg)

            qt2, ktp, vq4 = handles[b]
            emit_phase_b(
                nc, pools, Od, b, qt2, ktp, vq4, ones, wsel, ident2, prefetch, bg
            )
            while bg:
                bg.pop(0)()


_nc_cache = None


def build_nc():
    global _nc_cache
    if _nc_cache is not None:
        return _nc_cache
    nc = bacc.Bacc(None, target_bir_lowering=False)
    Qd = nc.declare_dram_parameter("Q", [BS, S, D], F32, isOutput=False)
    Kd = nc.declare_dram_parameter("K", [BS, S, D], F32, isOutput=False)
    Vd = nc.declare_dram_parameter("V", [BS, S, D], F32, isOutput=False)
    Od = nc.declare_dram_parameter("out", [BS, S, D], F32, isOutput=True)
    with TileContext(nc) as tc:
        build_body(nc, tc, Qd, Kd, Vd, Od)
    nc.finalize()
    _nc_cache = nc
    return nc


def kernel(Q, K, V):
    Q = np.asarray(Q, dtype=np.float32)
    K = np.asarray(K, dtype=np.float32)
    V = np.asarray(V, dtype=np.float32)
    nc = build_nc()
    in_maps = [
        {
            "Q": np.ascontiguousarray(Q[i * BS : (i + 1) * BS]),
            "K": np.ascontiguousarray(K[i * BS : (i + 1) * BS]),
            "V": np.ascontiguousarray(V[i * BS : (i + 1) * BS]),
        }
        for i in range(N_CORES)
    ]
    res = run_bass_kernel_spmd(nc, in_maps, core_ids=list(range(N_CORES)))
    return np.concatenate([res.results[i]["out"] for i in range(N_CORES)], axis=0)


# revision 30
# speedup vs baseline: 1.0945x; 1.0192x over previous
"""Distributed attention kernel for trn2 (8 NeuronCores).

Problem: B=16, S=4096, D=64 attention, out = softmax(Q K^T / sqrt(D)) V.
Sharding: batch dim B across 8 cores (2 batches per core), no collectives.

Per-core dataflow (everything in "transposed score" layout; PE assumed
pinned at 1.2 GHz, so all matmuls are packed with tile_position
concurrency):
  - K^T loaded via bf16 DRAM scratch + x-bar DMA transpose in an
    even/odd-s interleaved layout: ktp[128, 2048], top half = K^T of
    even s, bottom half = odd s.
  - Q^T duplicated onto both partition halves (qt2[128, 4096]) via
    doubled load + TensorE transposes, so 2x2-tiled score matmuls can
    source rhs from either half.
  - V loaded even/odd interleaved (vq[128, g, parity, 64]).
  - Per 512-wide q tile, per group g (256 consecutive k):
      S-quad: 4 concurrent K=64/M=64 matmuls -> S^T for even k (bank 0)
              and odd k (bank 1) of sp[128, 1024].
      exp:    one ScalarE activation [128, 1024] psum -> et bf16.
      AV-quad: 4 concurrent M=32 col-tiled matmuls accumulate
              O_even (ot2[0:64]) and O_odd (ot2[64:128]).
      sums:   every 2nd group, 4 concurrent M=1 matmuls with a ones
              vector accumulate sum(exp) into rows {0,32,64,96} of rs.
  - Phase C: copy to SBUF, accumulating PE transposes add the even/odd
    halves, a tiny matmul with a 4-hot selector vector folds the 4 sum
    rows into r[q], reciprocal + per-partition scale, contiguous DMA.
"""

import numpy as np

import concourse.bass as bass
import concourse.mybir as mybir
from concourse import bacc
from concourse.tile import TileContext
from concourse.bass_utils import run_bass_kernel_spmd
from concourse.masks import make_identity

B, S, D = 16, 4096, 64
N_CORES = 8
BS = B // N_CORES  # batches per core
SCALE = 1.0 / np.sqrt(D)  # 0.125
F32 = mybir.dt.float32
BF16 = mybir.dt.bfloat16

QTW = 512  # q-tile width
NG = S // 256  # 16 groups of 256 k (even/odd chunk pair) per q tile
N_QT = S // QTW  # 8 q tiles
NCH = S // 128  # 32 chunks of 128 rows


def emit_phase_a_alloc(nc, pools):
    # Layouts (k-permutation invariant):
    #   ktp[0:64, a]   = K[a, :]^T        for a in [0, 2048)   (k half 0)
    #   ktp[64:128, a] = K[2048 + a, :]^T                      (k half 1)
    #   qt2 = Q^T duplicated on both partition halves
    #   vq4[:, g, h, d]: row r = V[h * 2048 + g * 128 + r, d]
    qkp, vp = pools["qk"], pools["v"]
    qt2 = qkp.tile([128, S], BF16, tag="qt2")
    ktp = qkp.tile([128, 2048], BF16, tag="ktp")
    vq = vp.tile([128, NG * 2 * 64], BF16, tag="vq")
    vq4 = vq[:].rearrange("p (g h d) -> p g h d", h=2, d=64)
    return qt2, ktp, vq4


def emit_phase_a_k(nc, pools, Kd, b, identB, handles, piece, bg=None):
    natp, tpp = pools["nat"], pools["small"]
    qt2, ktp, vq4 = handles
    p0 = piece * 4  # 4 pairs per piece; pair pr = chunks (pr, 16+pr)
    knp = natp.tile([128, 4 * 2 * 64], BF16, tag=f"knat{piece % 2}", name="knp")
    kn4 = knp[:].rearrange("p (pr h d) -> p pr h d", h=2, d=64)
    for h in range(2):
        c0 = h * 16 + p0
        nc.gpsimd.dma_start(
            out=kn4[:, :, h, :],
            in_=Kd[b, c0 * 128 : (c0 + 4) * 128].rearrange(
                "(pr r) d -> r pr d", r=128
            ),
        )

    def work(i, trg, off=0):
        pr = p0 + i
        tr = trg[:, off * 128 : (off + 1) * 128]
        nc.tensor.transpose(tr, knp[:, i * 128 : (i + 1) * 128], identB[:])
        nc.vector.tensor_copy(ktp[:, pr * 128 : (pr + 1) * 128], tr[:])

    if bg is None:
        trg = tpp.tile([128, 4 * 128], BF16, tag="small", name="trgk")
        for i in range(4):
            work(i, trg, i)
    else:
        def closure(i):
            trg1 = tpp.tile([128, 128], BF16, tag="small", name="trg1")
            work(i, trg1, 0)

        for i in range(4):
            bg.append(lambda i=i: closure(i))


def emit_phase_a_q(nc, pools, Qd, b, identB, handles, piece, bg=None):
    natp, tpp = pools["nat"], pools["small"]
    qt2, ktp, vq4 = handles
    c0 = piece * 8
    qnat2 = natp.tile([128, 8 * 2 * 64], BF16, tag=f"qnat{piece % 2}")
    qn4 = qnat2[:].rearrange("p (c two d) -> p c two d", two=2, d=64)
    srcQ = Qd[b, c0 * 128 : (c0 + 8) * 128].rearrange("(c p) d -> p c d", p=128)
    nc.gpsimd.dma_start(out=qn4[:, :, 0, :], in_=srcQ)
    nc.gpsimd.dma_start(out=qn4[:, :, 1, :], in_=srcQ)
    def work(i, trg):
        c = c0 + i
        tr = trg[:, i * 128 : (i + 1) * 128]
        nc.tensor.transpose(tr, qnat2[:, i * 128 : (i + 1) * 128], identB[:])
        nc.vector.tensor_copy(qt2[:, c * 128 : (c + 1) * 128], tr[:])

    if bg is None:
        trg = tpp.tile([128, 8 * 128], BF16, tag="small")
        for i in range(8):
            work(i, trg)
    else:
        state = {}

        def closure(i):
            if "trg" not in state:
                state["trg"] = tpp.tile([128, 8 * 128], BF16, tag="small", name="trg")
            work(i, state["trg"])

        for i in range(8):
            bg.append(lambda i=i: closure(i))


def emit_phase_a_v(nc, pools, Vd, b, handles):
    qt2, ktp, vq4 = handles
    for h in range(2):
        nc.gpsimd.dma_start(
            out=vq4[:, :, h, :],
            in_=Vd[b, h * 2048 : (h + 1) * 2048].rearrange(
                "(g r) d -> r g d", r=128
            ),
        )


def emit_phase_b(nc, pools, Od, b, qt2, ktp, vq4, ones, wsel, ident2, after_qt0, bg):
    spp, opp, rsp, tpp, ep, fp = (
        pools["sp"], pools["ot"], pools["rs"], pools["small"],
        pools["et"], pools["fin"],
    )
    NGG = N_QT * NG  # 128 groups per batch
    sp_tiles = {}

    def emit_squad(gg):
        qi, g = gg // NG, gg % NG
        qc_lo = qt2[0:64, qi * QTW : (qi + 1) * QTW]
        qc_hi = qt2[64:128, qi * QTW : (qi + 1) * QTW]
        sp = spp.tile([128, QTW * 2], F32, tag="sp")
        sp_tiles[gg] = sp
        nc.tensor.matmul(
            sp[0:64, 0:QTW], ktp[0:64, g * 128 : g * 128 + 64], qc_lo,
            start=True, stop=True, skip_group_check=True,
        )
        nc.tensor.matmul(
            sp[64:128, 0:QTW], ktp[0:64, g * 128 + 64 : g * 128 + 128], qc_lo,
            start=True, stop=True, skip_group_check=True,
        )
        nc.tensor.matmul(
            sp[0:64, QTW : 2 * QTW], ktp[64:128, g * 128 : g * 128 + 64], qc_hi,
            start=True, stop=True, skip_group_check=True,
        )
        nc.tensor.matmul(
            sp[64:128, QTW : 2 * QTW],
            ktp[64:128, g * 128 + 64 : g * 128 + 128], qc_hi,
            start=True, stop=True, skip_group_check=True,
        )

    emit_squad(0)
    emit_squad(1)
    ot2 = rs = None
    et_prev = None
    for gg in range(NGG):
        qi, g = gg // NG, gg % NG
        if g == 0:
            ot2 = opp.tile([128, QTW], F32, tag="ot2")
            rs = rsp.tile([97, QTW], F32, tag="rs")
        sp = sp_tiles.pop(gg)
        et = ep.tile([128, QTW * 2], BF16, tag="et")
        nc.scalar.activation(
            et[:], sp[:], mybir.ActivationFunctionType.Exp, scale=SCALE
        )
        if gg + 2 < NGG:
            emit_squad(gg + 2)
        for t in range(4):
            par = t // 2
            nc.tensor.matmul(
                ot2[32 * t : 32 * (t + 1), :],
                vq4[:, g, par, 32 * (t % 2) : 32 * (t % 2 + 1)],
                et[:, par * QTW : (par + 1) * QTW],
                start=(g == 0), stop=(g == NG - 1), skip_group_check=True,
                tile_position=(0, 32 * t),
            )
        if g % 2 == 1:
            for t, (esrc, half) in enumerate(
                [(et_prev, 0), (et_prev, 1), (et, 0), (et, 1)]
            ):
                nc.tensor.matmul(
                    rs[32 * t : 32 * t + 1, :],
                    ones[:],
                    esrc[:, half * QTW : (half + 1) * QTW],
                    start=(g == 1), stop=(g == NG - 1),
                    skip_group_check=True, tile_position=(0, 32 * t),
                )
        et_prev = et
        for _ in range(3):
            if bg:
                bg.pop(0)()

        if g == NG - 1:
            # ---- Phase C for q-tile qi ----
            osb = fp.tile([128, QTW], BF16, tag="osb")
            nc.vector.tensor_copy(osb[:], ot2[:])
            rsb = fp.tile([97, QTW], BF16, tag="rsb")
            nc.vector.tensor_copy(rsb[:], rs[:])
            ctp = tpp.tile([128, 4 * 64], F32, tag="small")
            rcol = rsp.tile([128, 4], F32, tag="rs")
            for j in range(QTW // 128):
                js = slice(j * 128, (j + 1) * 128)
                nc.tensor.matmul(
                    ctp[:, j * 64 : (j + 1) * 64], osb[:, js], ident2[:],
                    start=True, stop=True, skip_group_check=True,
                )
                nc.tensor.matmul(
                    rcol[:, j : j + 1], rsb[:, js], wsel[:],
                    start=True, stop=True, skip_group_check=True,
                )
            rinv = fp.tile([128, 4], F32, tag="rinv")
            nc.vector.reciprocal(rinv[:], rcol[:])
            ob = fp.tile([128, 4 * 64], F32, tag="ob")
            for j in range(QTW // 128):
                nc.vector.tensor_scalar_mul(
                    ob[:, j * 64 : (j + 1) * 64],
                    ctp[:, j * 64 : (j + 1) * 64], rinv[:, j : j + 1]
                )
            nc.sync.dma_start(
                out=Od[b, qi * QTW : (qi + 1) * QTW].rearrange(
                    "(j p) d -> p j d", p=128
                ),
                in_=ob[:].rearrange("p (j d) -> p j d", d=64),
            )
            if after_qt0 is not None:
                after_qt0(qi)


def build_body(nc, tc, Qd, Kd, Vd, Od):
    with (
        tc.tile_pool(name="const", bufs=1) as constp,
        tc.tile_pool(name="qk", bufs=2) as qkp,
        tc.tile_pool(name="v", bufs=2) as vp,
        tc.tile_pool(name="nat", bufs=2) as natp,
        tc.tile_pool(name="sp", bufs=2, space="PSUM") as spp,
        tc.tile_pool(name="ot", bufs=1, space="PSUM") as opp,
        tc.tile_pool(name="rs", bufs=1, space="PSUM") as rsp,
        tc.tile_pool(name="small", bufs=2, space="PSUM") as tpp,
        tc.tile_pool(name="et", bufs=6) as ep,
        tc.tile_pool(name="fin", bufs=4) as fp,
    ):
        pools = {
            "qk": qkp, "v": vp, "nat": natp, "sp": spp,
            "ot": opp, "rs": rsp, "small": tpp, "et": ep, "fin": fp,
        }
        ident2 = constp.tile([128, 64], BF16)
        nc.gpsimd.memset(ident2[:], 0.0)
        for half in range(2):
            nc.gpsimd.affine_select(
                out=ident2[64 * half : 64 * (half + 1), :],
                in_=ident2[64 * half : 64 * (half + 1), :],
                compare_op=mybir.AluOpType.not_equal, fill=1.0, base=0,
                pattern=[[-1, 64]], channel_multiplier=1,
            )
        identB = constp.tile([128, 128], BF16)
        make_identity(nc, identB[:])
        ones = constp.tile([128, 1], BF16)
        nc.gpsimd.memset(ones[:], 1.0)
        wsel = constp.tile([97, 1], BF16)
        nc.gpsimd.memset(wsel[:], 0.0)
        for t in range(4):
            nc.gpsimd.memset(wsel[32 * t : 32 * t + 1, :], 1.0)

        handles = [None] * BS
        bg = []
        handles[0] = emit_phase_a_alloc(nc, pools)
        for piece in (0, 1):
            emit_phase_a_k(nc, pools, Kd, 0, identB, handles[0], piece)
        emit_phase_a_q(nc, pools, Qd, 0, identB, handles[0], 0)
        emit_phase_a_v(nc, pools, Vd, 0, handles[0])
        for piece in (2, 3):
            emit_phase_a_k(nc, pools, Kd, 0, identB, handles[0], piece, bg)
        for piece in (1, 2, 3):
            emit_phase_a_q(nc, pools, Qd, 0, identB, handles[0], piece, bg)
        for b in range(BS):

            def prefetch(qi, b=b, bg=bg):
                if b + 1 >= BS:
                    return
                if qi == 0:
                    handles[b + 1] = emit_phase_a_alloc(nc, pools)
                    emit_phase_a_k(nc, pools, Kd, b + 1, identB, handles[b + 1], 0, bg)
                elif qi == 1:
                    emit_phase_a_k(nc, pools, Kd, b + 1, identB, handles[b + 1], 2, bg)
                    emit_phase_a_q(nc, pools, Qd, b + 1, identB, handles[b + 1], 0, bg)
                elif qi == 2:
                    emit_phase_a_v(nc, pools, Vd, b + 1, handles[b + 1])
                    emit_phase_a_k(nc, pools, Kd, b + 1, identB, handles[b + 1], 1, bg)
                elif qi == 3:
                    emit_phase_a_k(nc, pools, Kd, b + 1, identB, handles[b + 1], 3, bg)
                    emit_phase_a_q(nc, pools, Qd, b + 1, identB, handles[b + 1], 1, bg)
                elif qi == 4:
                    emit_phase_a_q(nc, pools, Qd, b + 1, identB, handles[b + 1], 2, bg)
                elif qi == 5:
                    emit_phase_a_q(nc, pools, Qd, b + 1, identB, handles[b + 1], 3, bg)

            qt2, ktp, vq4 = handles[b]
            emit_phase_b(
                nc, pools, Od, b, qt2, ktp, vq4, ones, wsel, ident2, prefetch, bg
            )
            while bg:
                bg.pop(0)()


_nc_cache = None


def build_nc():
    global _nc_cache
    if _nc_cache is not None:
        return _nc_cache
    nc = bacc.Bacc(None, target_bir_lowering=False)
    Qd = nc.declare_dram_parameter("Q", [BS, S, D], F32, isOutput=False)
    Kd = nc.declare_dram_parameter("K", [BS, S, D], F32, isOutput=False)
    Vd = nc.declare_dram_parameter("V", [BS, S, D], F32, isOutput=False)
    Od = nc.declare_dram_parameter("out", [BS, S, D], F32, isOutput=True)
    with TileContext(nc) as tc:
        build_body(nc, tc, Qd, Kd, Vd, Od)
    nc.finalize()
    _nc_cache = nc
    return nc


def kernel(Q, K, V):
    Q = np.asarray(Q, dtype=np.float32)
    K = np.asarray(K, dtype=np.float32)
    V = np.asarray(V, dtype=np.float32)
    nc = build_nc()
    in_maps = [
        {
            "Q": np.ascontiguousarray(Q[i * BS : (i + 1) * BS]),
            "K": np.ascontiguousarray(K[i * BS : (i + 1) * BS]),
            "V": np.ascontiguousarray(V[i * BS : (i + 1) * BS]),
        }
        for i in range(N_CORES)
    ]
    res = run_bass_kernel_spmd(nc, in_maps, core_ids=list(range(N_CORES)))
    return np.concatenate([res.results[i]["out"] for i in range(N_CORES)], axis=0)


# revision 31
# speedup vs baseline: 1.1589x; 1.0589x over previous
"""Distributed attention kernel for trn2 (8 NeuronCores).

Problem: B=16, S=4096, D=64 attention, out = softmax(Q K^T / sqrt(D)) V.
Sharding: batch dim B across 8 cores (2 batches per core), no collectives.

Per-core dataflow (everything in "transposed score" layout; PE assumed
pinned at 1.2 GHz, so all matmuls are packed with tile_position
concurrency):
  - K^T loaded via bf16 DRAM scratch + x-bar DMA transpose in an
    even/odd-s interleaved layout: ktp[128, 2048], top half = K^T of
    even s, bottom half = odd s.
  - Q^T duplicated onto both partition halves (qt2[128, 4096]) via
    doubled load + TensorE transposes, so 2x2-tiled score matmuls can
    source rhs from either half.
  - V loaded even/odd interleaved (vq[128, g, parity, 64]).
  - Per 512-wide q tile, per group g (256 consecutive k):
      S-quad: 4 concurrent K=64/M=64 matmuls -> S^T for even k (bank 0)
              and odd k (bank 1) of sp[128, 1024].
      exp:    one ScalarE activation [128, 1024] psum -> et bf16.
      AV-quad: 4 concurrent M=32 col-tiled matmuls accumulate
              O_even (ot2[0:64]) and O_odd (ot2[64:128]).
      sums:   every 2nd group, 4 concurrent M=1 matmuls with a ones
              vector accumulate sum(exp) into rows {0,32,64,96} of rs.
  - Phase C: copy to SBUF, accumulating PE transposes add the even/odd
    halves, a tiny matmul with a 4-hot selector vector folds the 4 sum
    rows into r[q], reciprocal + per-partition scale, contiguous DMA.
"""

import numpy as np

import concourse.bass as bass
import concourse.mybir as mybir
from concourse import bacc
from concourse.tile import TileContext
from concourse.bass_utils import run_bass_kernel_spmd
from concourse.masks import make_identity

B, S, D = 16, 4096, 64
N_CORES = 8
BS = B // N_CORES  # batches per core
SCALE = 1.0 / np.sqrt(D)  # 0.125
F32 = mybir.dt.float32
BF16 = mybir.dt.bfloat16

QTW = 512  # q-tile width
NG = S // 256  # 16 groups of 256 k (even/odd chunk pair) per q tile
N_QT = S // QTW  # 8 q tiles
NCH = S // 128  # 32 chunks of 128 rows


def emit_phase_a_alloc(nc, pools):
    # Layouts (k-permutation invariant):
    #   ktp[0:64, a]   = K[a, :]^T        for a in [0, 2048)   (k half 0)
    #   ktp[64:128, a] = K[2048 + a, :]^T                      (k half 1)
    #   qt2 = Q^T duplicated on both partition halves
    #   vq4[:, g, h, d]: row r = V[h * 2048 + g * 128 + r, d]
    qkp, vp = pools["qk"], pools["v"]
    qt2 = qkp.tile([128, S], BF16, tag="qt2")
    ktp = qkp.tile([128, 2048], BF16, tag="ktp")
    vq = vp.tile([128, NG * 2 * 64], BF16, tag="vq")
    vq4 = vq[:].rearrange("p (g h d) -> p g h d", h=2, d=64)
    return qt2, ktp, vq4


def emit_phase_a_k(nc, pools, Kd, b, identB, handles, piece, bg=None):
    natp, tpp = pools["nat"], pools["small"]
    qt2, ktp, vq4 = handles
    p0 = piece * 4  # 4 pairs per piece; pair pr = chunks (pr, 16+pr)
    knp = natp.tile([128, 4 * 2 * 64], BF16, tag=f"knat{piece % 2}", name="knp")
    kn4 = knp[:].rearrange("p (pr h d) -> p pr h d", h=2, d=64)
    for h in range(2):
        c0 = h * 16 + p0
        nc.gpsimd.dma_start(
            out=kn4[:, :, h, :],
            in_=Kd[b, c0 * 128 : (c0 + 4) * 128].rearrange(
                "(pr r) d -> r pr d", r=128
            ),
        )

    def work(i, trg, off=0):
        pr = p0 + i
        tr = trg[:, off * 128 : (off + 1) * 128]
        nc.tensor.transpose(tr, knp[:, i * 128 : (i + 1) * 128], identB[:])
        nc.vector.tensor_copy(ktp[:, pr * 128 : (pr + 1) * 128], tr[:])

    if bg is None:
        trg = tpp.tile([128, 4 * 128], BF16, tag="small", name="trgk")
        for i in range(4):
            work(i, trg, i)
    else:
        def closure(i):
            trg1 = tpp.tile([128, 128], BF16, tag="small", name="trg1")
            work(i, trg1, 0)

        for i in range(4):
            bg.append(lambda i=i: closure(i))


def emit_phase_a_q(nc, pools, Qd, b, identB, handles, piece, bg=None):
    natp, tpp = pools["nat"], pools["small"]
    qt2, ktp, vq4 = handles
    c0 = piece * 8
    qnat2 = natp.tile([128, 8 * 2 * 64], BF16, tag=f"qnat{piece % 2}")
    qn4 = qnat2[:].rearrange("p (c two d) -> p c two d", two=2, d=64)
    srcQ = Qd[b, c0 * 128 : (c0 + 8) * 128].rearrange("(c p) d -> p c d", p=128)
    nc.gpsimd.dma_start(out=qn4[:, :, 0, :], in_=srcQ)
    nc.gpsimd.dma_start(out=qn4[:, :, 1, :], in_=srcQ)
    def work(i, trg):
        c = c0 + i
        tr = trg[:, i * 128 : (i + 1) * 128]
        nc.tensor.transpose(tr, qnat2[:, i * 128 : (i + 1) * 128], identB[:])
        nc.vector.tensor_copy(qt2[:, c * 128 : (c + 1) * 128], tr[:])

    if bg is None:
        trg = tpp.tile([128, 8 * 128], BF16, tag="small")
        for i in range(8):
            work(i, trg)
    else:
        state = {}

        def closure(i):
            if "trg" not in state:
                state["trg"] = tpp.tile([128, 8 * 128], BF16, tag="small", name="trg")
            work(i, state["trg"])

        for i in range(8):
            bg.append(lambda i=i: closure(i))


def emit_phase_a_v(nc, pools, Vd, b, handles):
    qt2, ktp, vq4 = handles
    for h in range(2):
        nc.gpsimd.dma_start(
            out=vq4[:, :, h, :],
            in_=Vd[b, h * 2048 : (h + 1) * 2048].rearrange(
                "(g r) d -> r g d", r=128
            ),
        )


def emit_phase_b(nc, pools, Od, b, qt2, ktp, vq4, ones, wsel, ident2, after_qt0, bg):
    spp, opp, rsp, tpp, ep, fp = (
        pools["sp"], pools["ot"], pools["rs"], pools["small"],
        pools["et"], pools["fin"],
    )
    NGG = N_QT * NG  # 128 groups per batch
    sp_tiles = {}

    def emit_squad(gg):
        qi, g = gg // NG, gg % NG
        qc_lo = qt2[0:64, qi * QTW : (qi + 1) * QTW]
        qc_hi = qt2[64:128, qi * QTW : (qi + 1) * QTW]
        sp = spp.tile([128, QTW * 2], F32, tag="sp")
        sp_tiles[gg] = sp
        nc.tensor.matmul(
            sp[0:64, 0:QTW], ktp[0:64, g * 128 : g * 128 + 64], qc_lo,
            start=True, stop=True, skip_group_check=True,
        )
        nc.tensor.matmul(
            sp[64:128, 0:QTW], ktp[0:64, g * 128 + 64 : g * 128 + 128], qc_lo,
            start=True, stop=True, skip_group_check=True,
        )
        nc.tensor.matmul(
            sp[0:64, QTW : 2 * QTW], ktp[64:128, g * 128 : g * 128 + 64], qc_hi,
            start=True, stop=True, skip_group_check=True,
        )
        nc.tensor.matmul(
            sp[64:128, QTW : 2 * QTW],
            ktp[64:128, g * 128 + 64 : g * 128 + 128], qc_hi,
            start=True, stop=True, skip_group_check=True,
        )

    emit_squad(0)
    emit_squad(1)
    ot2 = rs = None
    et_prev = None
    for gg in range(NGG):
        qi, g = gg // NG, gg % NG
        if g == 0:
            ot2 = opp.tile([128, QTW], F32, tag="ot2")
            rs = rsp.tile([97, QTW], F32, tag="rs")
        sp = sp_tiles.pop(gg)
        et = ep.tile([128, QTW * 2], BF16, tag="et")
        nc.scalar.activation(
            et[:], sp[:], mybir.ActivationFunctionType.Exp, scale=SCALE
        )
        if gg + 2 < NGG:
            emit_squad(gg + 2)
        for t in range(4):
            par = t // 2
            nc.tensor.matmul(
                ot2[32 * t : 32 * (t + 1), :],
                vq4[:, g, par, 32 * (t % 2) : 32 * (t % 2 + 1)],
                et[:, par * QTW : (par + 1) * QTW],
                start=(g == 0), stop=(g == NG - 1), skip_group_check=True,
                tile_position=(0, 32 * t),
            )
        if g % 2 == 1:
            for t, (esrc, half) in enumerate(
                [(et_prev, 0), (et_prev, 1), (et, 0), (et, 1)]
            ):
                nc.tensor.matmul(
                    rs[32 * t : 32 * t + 1, :],
                    ones[:],
                    esrc[:, half * QTW : (half + 1) * QTW],
                    start=(g == 1), stop=(g == NG - 1),
                    skip_group_check=True, tile_position=(0, 32 * t),
                )
        et_prev = et
        if bg:
            bg.pop(0)()

        if g == NG - 1:
            # ---- Phase C for q-tile qi ----
            osb = fp.tile([128, QTW], BF16, tag="osb")
            nc.vector.tensor_copy(osb[:], ot2[:])
            rsb = fp.tile([97, QTW], BF16, tag="rsb")
            nc.vector.tensor_copy(rsb[:], rs[:])
            ctp = tpp.tile([128, 4 * 64], F32, tag="small")
            rcol = rsp.tile([128, 4], F32, tag="rs")
            for j in range(QTW // 128):
                js = slice(j * 128, (j + 1) * 128)
                nc.tensor.matmul(
                    ctp[:, j * 64 : (j + 1) * 64], osb[:, js], ident2[:],
                    start=True, stop=True, skip_group_check=True,
                )
                nc.tensor.matmul(
                    rcol[:, j : j + 1], rsb[:, js], wsel[:],
                    start=True, stop=True, skip_group_check=True,
                )
            rinv = fp.tile([128, 4], F32, tag="rinv")
            nc.vector.reciprocal(rinv[:], rcol[:])
            ob = fp.tile([128, 4 * 64], F32, tag="ob")
            for j in range(QTW // 128):
                nc.vector.tensor_scalar_mul(
                    ob[:, j * 64 : (j + 1) * 64],
                    ctp[:, j * 64 : (j + 1) * 64], rinv[:, j : j + 1]
                )
            nc.sync.dma_start(
                out=Od[b, qi * QTW : (qi + 1) * QTW].rearrange(
                    "(j p) d -> p j d", p=128
                ),
                in_=ob[:].rearrange("p (j d) -> p j d", d=64),
            )
            if after_qt0 is not None:
                after_qt0(qi)


def build_body(nc, tc, Qd, Kd, Vd, Od):
    with (
        tc.tile_pool(name="const", bufs=1) as constp,
        tc.tile_pool(name="qk", bufs=2) as qkp,
        tc.tile_pool(name="v", bufs=2) as vp,
        tc.tile_pool(name="nat", bufs=2) as natp,
        tc.tile_pool(name="sp", bufs=2, space="PSUM") as spp,
        tc.tile_pool(name="ot", bufs=1, space="PSUM") as opp,
        tc.tile_pool(name="rs", bufs=1, space="PSUM") as rsp,
        tc.tile_pool(name="small", bufs=2, space="PSUM") as tpp,
        tc.tile_pool(name="et", bufs=6) as ep,
        tc.tile_pool(name="fin", bufs=4) as fp,
    ):
        pools = {
            "qk": qkp, "v": vp, "nat": natp, "sp": spp,
            "ot": opp, "rs": rsp, "small": tpp, "et": ep, "fin": fp,
        }
        ident2 = constp.tile([128, 64], BF16)
        nc.gpsimd.memset(ident2[:], 0.0)
        for half in range(2):
            nc.gpsimd.affine_select(
                out=ident2[64 * half : 64 * (half + 1), :],
                in_=ident2[64 * half : 64 * (half + 1), :],
                compare_op=mybir.AluOpType.not_equal, fill=1.0, base=0,
                pattern=[[-1, 64]], channel_multiplier=1,
            )
        identB = constp.tile([128, 128], BF16)
        make_identity(nc, identB[:])
        ones = constp.tile([128, 1], BF16)
        nc.gpsimd.memset(ones[:], 1.0)
        wsel = constp.tile([97, 1], BF16)
        nc.gpsimd.memset(wsel[:], 0.0)
        for t in range(4):
            nc.gpsimd.memset(wsel[32 * t : 32 * t + 1, :], 1.0)

        handles = [None] * BS
        bg = []
        handles[0] = emit_phase_a_alloc(nc, pools)
        for piece in (0, 1):
            emit_phase_a_k(nc, pools, Kd, 0, identB, handles[0], piece)
        emit_phase_a_q(nc, pools, Qd, 0, identB, handles[0], 0)
        emit_phase_a_v(nc, pools, Vd, 0, handles[0])
        for piece in (2, 3):
            emit_phase_a_k(nc, pools, Kd, 0, identB, handles[0], piece, bg)
        for piece in (1, 2, 3):
            emit_phase_a_q(nc, pools, Qd, 0, identB, handles[0], piece, bg)
        for b in range(BS):

            def prefetch(qi, b=b, bg=bg):
                if b + 1 >= BS:
                    return
                if qi == 0:
                    handles[b + 1] = emit_phase_a_alloc(nc, pools)
                    emit_phase_a_k(nc, pools, Kd, b + 1, identB, handles[b + 1], 0, bg)
                elif qi == 1:
                    emit_phase_a_k(nc, pools, Kd, b + 1, identB, handles[b + 1], 2, bg)
                    emit_phase_a_q(nc, pools, Qd, b + 1, identB, handles[b + 1], 0, bg)
                elif qi == 2:
                    emit_phase_a_v(nc, pools, Vd, b + 1, handles[b + 1])
                    emit_phase_a_k(nc, pools, Kd, b + 1, identB, handles[b + 1], 1, bg)
                elif qi == 3:
                    emit_phase_a_k(nc, pools, Kd, b + 1, identB, handles[b + 1], 3, bg)
                    emit_phase_a_q(nc, pools, Qd, b + 1, identB, handles[b + 1], 1, bg)
                elif qi == 4:
                    emit_phase_a_q(nc, pools, Qd, b + 1, identB, handles[b + 1], 2, bg)
                elif qi == 5:
                    emit_phase_a_q(nc, pools, Qd, b + 1, identB, handles[b + 1], 3, bg)

            qt2, ktp, vq4 = handles[b]
            emit_phase_b(
                nc, pools, Od, b, qt2, ktp, vq4, ones, wsel, ident2, prefetch, bg
            )
            while bg:
                bg.pop(0)()


_nc_cache = None


def build_nc():
    global _nc_cache
    if _nc_cache is not None:
        return _nc_cache
    nc = bacc.Bacc(None, target_bir_lowering=False)
    Qd = nc.declare_dram_parameter("Q", [BS, S, D], F32, isOutput=False)
    Kd = nc.declare_dram_parameter("K", [BS, S, D], F32, isOutput=False)
    Vd = nc.declare_dram_parameter("V", [BS, S, D], F32, isOutput=False)
    Od = nc.declare_dram_parameter("out", [BS, S, D], F32, isOutput=True)
    with TileContext(nc) as tc:
        build_body(nc, tc, Qd, Kd, Vd, Od)
    nc.finalize()
    _nc_cache = nc
    return nc


def kernel(Q, K, V):
    Q = np.asarray(Q, dtype=np.float32)
    K = np.asarray(K, dtype=np.float32)
    V = np.asarray(V, dtype=np.float32)
    nc = build_nc()
    in_maps = [
        {
            "Q": np.ascontiguousarray(Q[i * BS : (i + 1) * BS]),
            "K": np.ascontiguousarray(K[i * BS : (i + 1) * BS]),
            "V": np.ascontiguousarray(V[i * BS : (i + 1) * BS]),
        }
        for i in range(N_CORES)
    ]
    res = run_bass_kernel_spmd(nc, in_maps, core_ids=list(range(N_CORES)))
    return np.concatenate([res.results[i]["out"] for i in range(N_CORES)], axis=0)


# revision 32
# speedup vs baseline: 1.1684x; 1.0081x over previous
"""Distributed attention kernel for trn2 (8 NeuronCores).

Problem: B=16, S=4096, D=64 attention, out = softmax(Q K^T / sqrt(D)) V.
Sharding: batch dim B across 8 cores (2 batches per core), no collectives.

Per-core dataflow (everything in "transposed score" layout; PE assumed
pinned at 1.2 GHz, so all matmuls are packed with tile_position
concurrency):
  - K^T loaded via bf16 DRAM scratch + x-bar DMA transpose in an
    even/odd-s interleaved layout: ktp[128, 2048], top half = K^T of
    even s, bottom half = odd s.
  - Q^T duplicated onto both partition halves (qt2[128, 4096]) via
    doubled load + TensorE transposes, so 2x2-tiled score matmuls can
    source rhs from either half.
  - V loaded even/odd interleaved (vq[128, g, parity, 64]).
  - Per 512-wide q tile, per group g (256 consecutive k):
      S-quad: 4 concurrent K=64/M=64 matmuls -> S^T for even k (bank 0)
              and odd k (bank 1) of sp[128, 1024].
      exp:    one ScalarE activation [128, 1024] psum -> et bf16.
      AV-quad: 4 concurrent M=32 col-tiled matmuls accumulate
              O_even (ot2[0:64]) and O_odd (ot2[64:128]).
      sums:   every 2nd group, 4 concurrent M=1 matmuls with a ones
              vector accumulate sum(exp) into rows {0,32,64,96} of rs.
  - Phase C: copy to SBUF, accumulating PE transposes add the even/odd
    halves, a tiny matmul with a 4-hot selector vector folds the 4 sum
    rows into r[q], reciprocal + per-partition scale, contiguous DMA.
"""

import numpy as np

import concourse.bass as bass
import concourse.mybir as mybir
from concourse import bacc
from concourse.tile import TileContext
from concourse.bass_utils import run_bass_kernel_spmd
from concourse.masks import make_identity

B, S, D = 16, 4096, 64
N_CORES = 8
BS = B // N_CORES  # batches per core
SCALE = 1.0 / np.sqrt(D)  # 0.125
F32 = mybir.dt.float32
BF16 = mybir.dt.bfloat16

QTW = 512  # q-tile width
NG = S // 256  # 16 groups of 256 k (even/odd chunk pair) per q tile
N_QT = S // QTW  # 8 q tiles
NCH = S // 128  # 32 chunks of 128 rows


def emit_phase_a_alloc(nc, pools):
    # Layouts (k-permutation invariant):
    #   ktp[0:64, a]   = K[a, :]^T        for a in [0, 2048)   (k half 0)
    #   ktp[64:128, a] = K[2048 + a, :]^T                      (k half 1)
    #   qt2 = Q^T duplicated on both partition halves
    #   vq4[:, g, h, d]: row r = V[h * 2048 + g * 128 + r, d]
    qkp, vp = pools["qk"], pools["v"]
    qt2 = qkp.tile([128, S], BF16, tag="qt2")
    ktp = qkp.tile([128, 2048], BF16, tag="ktp")
    vq = vp.tile([128, NG * 2 * 64], BF16, tag="vq")
    vq4 = vq[:].rearrange("p (g h d) -> p g h d", h=2, d=64)
    return qt2, ktp, vq4


def emit_phase_a_k(nc, pools, Kd, b, identB, handles, piece, bg=None):
    natp, tpp = pools["nat"], pools["small"]
    qt2, ktp, vq4 = handles
    p0 = piece * 4  # 4 pairs per piece; pair pr = chunks (pr, 16+pr)
    knp = natp.tile([128, 4 * 2 * 64], BF16, tag=f"knat{piece % 2}", name="knp")
    kn4 = knp[:].rearrange("p (pr h d) -> p pr h d", h=2, d=64)
    for h in range(2):
        c0 = h * 16 + p0
        nc.gpsimd.dma_start(
            out=kn4[:, :, h, :],
            in_=Kd[b, c0 * 128 : (c0 + 4) * 128].rearrange(
                "(pr r) d -> r pr d", r=128
            ),
        )

    def work(i, trg, off=0):
        pr = p0 + i
        tr = trg[:, off * 128 : (off + 1) * 128]
        nc.tensor.transpose(tr, knp[:, i * 128 : (i + 1) * 128], identB[:])
        nc.vector.tensor_copy(ktp[:, pr * 128 : (pr + 1) * 128], tr[:])

    if bg is None:
        trg = tpp.tile([128, 4 * 128], BF16, tag="small", name="trgk")
        for i in range(4):
            work(i, trg, i)
    else:
        def closure(i):
            trg1 = tpp.tile([128, 128], BF16, tag="small", name="trg1")
            work(i, trg1, 0)

        for i in range(4):
            bg.append(lambda i=i: closure(i))


def emit_phase_a_q(nc, pools, Qd, b, identB, handles, piece, bg=None):
    natp, tpp = pools["nat"], pools["small"]
    qt2, ktp, vq4 = handles
    c0 = piece * 8
    qnat2 = natp.tile([128, 8 * 2 * 64], BF16, tag=f"qnat{piece % 2}")
    qn4 = qnat2[:].rearrange("p (c two d) -> p c two d", two=2, d=64)
    srcQ = Qd[b, c0 * 128 : (c0 + 8) * 128].rearrange("(c p) d -> p c d", p=128)
    nc.gpsimd.dma_start(out=qn4[:, :, 0, :], in_=srcQ)
    nc.gpsimd.dma_start(out=qn4[:, :, 1, :], in_=srcQ)
    def work(i, trg):
        c = c0 + i
        tr = trg[:, i * 128 : (i + 1) * 128]
        nc.tensor.transpose(tr, qnat2[:, i * 128 : (i + 1) * 128], identB[:])
        nc.vector.tensor_copy(qt2[:, c * 128 : (c + 1) * 128], tr[:])

    if bg is None:
        trg = tpp.tile([128, 8 * 128], BF16, tag="small")
        for i in range(8):
            work(i, trg)
    else:
        state = {}

        def closure(i):
            if "trg" not in state:
                state["trg"] = tpp.tile([128, 8 * 128], BF16, tag="small", name="trg")
            work(i, state["trg"])

        for i in range(8):
            bg.append(lambda i=i: closure(i))


def emit_phase_a_v(nc, pools, Vd, b, handles):
    qt2, ktp, vq4 = handles
    for h in range(2):
        nc.gpsimd.dma_start(
            out=vq4[:, :, h, :],
            in_=Vd[b, h * 2048 : (h + 1) * 2048].rearrange(
                "(g r) d -> r g d", r=128
            ),
        )


def emit_phase_b(nc, pools, Od, b, qt2, ktp, vq4, ones, wsel, ident2, after_qt0, bg):
    spp, opp, rsp, tpp, ep, fp = (
        pools["sp"], pools["ot"], pools["rs"], pools["small"],
        pools["et"], pools["fin"],
    )
    NGG = N_QT * NG  # 128 groups per batch
    sp_tiles = {}

    def emit_squad(gg):
        qi, g = gg // NG, gg % NG
        qc_lo = qt2[0:64, qi * QTW : (qi + 1) * QTW]
        qc_hi = qt2[64:128, qi * QTW : (qi + 1) * QTW]
        sp = spp.tile([128, QTW * 2], F32, tag="sp")
        sp_tiles[gg] = sp
        nc.tensor.matmul(
            sp[0:64, 0:QTW], ktp[0:64, g * 128 : g * 128 + 64], qc_lo,
            start=True, stop=True, skip_group_check=True,
        )
        nc.tensor.matmul(
            sp[64:128, 0:QTW], ktp[0:64, g * 128 + 64 : g * 128 + 128], qc_lo,
            start=True, stop=True, skip_group_check=True,
        )
        nc.tensor.matmul(
            sp[0:64, QTW : 2 * QTW], ktp[64:128, g * 128 : g * 128 + 64], qc_hi,
            start=True, stop=True, skip_group_check=True,
        )
        nc.tensor.matmul(
            sp[64:128, QTW : 2 * QTW],
            ktp[64:128, g * 128 + 64 : g * 128 + 128], qc_hi,
            start=True, stop=True, skip_group_check=True,
        )

    emit_squad(0)
    emit_squad(1)
    ot2 = rs = None
    et_prev = None
    for gg in range(NGG):
        qi, g = gg // NG, gg % NG
        if g == 0:
            ot2 = opp.tile([128, QTW], F32, tag="ot2")
            rs = rsp.tile([97, QTW], F32, tag="rs")
        sp = sp_tiles.pop(gg)
        et = ep.tile([128, QTW * 2], BF16, tag="et")
        nc.scalar.activation(
            et[:], sp[:], mybir.ActivationFunctionType.Exp, scale=SCALE
        )
        if gg + 2 < NGG:
            emit_squad(gg + 2)
        for t in range(4):
            par = t // 2
            nc.tensor.matmul(
                ot2[32 * t : 32 * (t + 1), :],
                vq4[:, g, par, 32 * (t % 2) : 32 * (t % 2 + 1)],
                et[:, par * QTW : (par + 1) * QTW],
                start=(g == 0), stop=(g == NG - 1), skip_group_check=True,
                tile_position=(0, 32 * t),
            )
        if g % 2 == 1:
            for t, (esrc, half) in enumerate(
                [(et_prev, 0), (et_prev, 1), (et, 0), (et, 1)]
            ):
                nc.tensor.matmul(
                    rs[32 * t : 32 * t + 1, :],
                    ones[:],
                    esrc[:, half * QTW : (half + 1) * QTW],
                    start=(g == 1), stop=(g == NG - 1),
                    skip_group_check=True, tile_position=(0, 32 * t),
                )
        et_prev = et
        if bg:
            bg.pop(0)()

        if g == NG - 1:
            # ---- Phase C for q-tile qi ----
            osb = fp.tile([128, QTW], BF16, tag="osb")
            nc.vector.tensor_copy(osb[:], ot2[:])
            rsb = fp.tile([97, QTW], BF16, tag="rsb")
            nc.vector.tensor_copy(rsb[:], rs[:])
            ctp = tpp.tile([128, 4 * 64], F32, tag="small")
            rcol = rsp.tile([128, 4], F32, tag="rs")
            for j in range(QTW // 128):
                js = slice(j * 128, (j + 1) * 128)
                nc.tensor.matmul(
                    ctp[:, j * 64 : (j + 1) * 64], osb[:, js], ident2[:],
                    start=True, stop=True, skip_group_check=True,
                )
                nc.tensor.matmul(
                    rcol[:, j : j + 1], rsb[:, js], wsel[:],
                    start=True, stop=True, skip_group_check=True,
                )
            rinv = fp.tile([128, 4], F32, tag="rinv")
            nc.vector.reciprocal(rinv[:], rcol[:])
            ob = fp.tile([128, 4 * 64], F32, tag="ob")
            for j in range(QTW // 128):
                nc.vector.tensor_scalar_mul(
                    ob[:, j * 64 : (j + 1) * 64],
                    ctp[:, j * 64 : (j + 1) * 64], rinv[:, j : j + 1]
                )
            nc.sync.dma_start(
                out=Od[b, qi * QTW : (qi + 1) * QTW].rearrange(
                    "(j p) d -> p j d", p=128
                ),
                in_=ob[:].rearrange("p (j d) -> p j d", d=64),
            )
            if after_qt0 is not None:
                after_qt0(qi)


def build_body(nc, tc, Qd, Kd, Vd, Od):
    with (
        tc.tile_pool(name="const", bufs=1) as constp,
        tc.tile_pool(name="qk", bufs=2) as qkp,
        tc.tile_pool(name="v", bufs=2) as vp,
        tc.tile_pool(name="nat", bufs=2) as natp,
        tc.tile_pool(name="sp", bufs=2, space="PSUM") as spp,
        tc.tile_pool(name="ot", bufs=1, space="PSUM") as opp,
        tc.tile_pool(name="rs", bufs=1, space="PSUM") as rsp,
        tc.tile_pool(name="small", bufs=2, space="PSUM") as tpp,
        tc.tile_pool(name="et", bufs=6) as ep,
        tc.tile_pool(name="fin", bufs=4) as fp,
    ):
        pools = {
            "qk": qkp, "v": vp, "nat": natp, "sp": spp,
            "ot": opp, "rs": rsp, "small": tpp, "et": ep, "fin": fp,
        }
        ident2 = constp.tile([128, 64], BF16)
        nc.gpsimd.memset(ident2[:], 0.0)
        for half in range(2):
            nc.gpsimd.affine_select(
                out=ident2[64 * half : 64 * (half + 1), :],
                in_=ident2[64 * half : 64 * (half + 1), :],
                compare_op=mybir.AluOpType.not_equal, fill=1.0, base=0,
                pattern=[[-1, 64]], channel_multiplier=1,
            )
        identB = constp.tile([128, 128], BF16)
        make_identity(nc, identB[:])
        ones = constp.tile([128, 1], BF16)
        nc.gpsimd.memset(ones[:], 1.0)
        warm = constp.tile([1, 1], F32)
        nc.scalar.activation(
            warm[:], ones[0:1, 0:1], mybir.ActivationFunctionType.Exp, scale=1.0
        )
        wsel = constp.tile([97, 1], BF16)
        nc.gpsimd.memset(wsel[:], 0.0)
        for t in range(4):
            nc.gpsimd.memset(wsel[32 * t : 32 * t + 1, :], 1.0)

        handles = [None] * BS
        bg = []
        handles[0] = emit_phase_a_alloc(nc, pools)
        for piece in (0, 1):
            emit_phase_a_k(nc, pools, Kd, 0, identB, handles[0], piece)
        emit_phase_a_q(nc, pools, Qd, 0, identB, handles[0], 0)
        emit_phase_a_v(nc, pools, Vd, 0, handles[0])
        for piece in (2, 3):
            emit_phase_a_k(nc, pools, Kd, 0, identB, handles[0], piece, bg)
        for piece in (1, 2, 3):
            emit_phase_a_q(nc, pools, Qd, 0, identB, handles[0], piece, bg)
        for b in range(BS):

            def prefetch(qi, b=b, bg=bg):
                if b + 1 >= BS:
                    return
                if qi == 0:
                    handles[b + 1] = emit_phase_a_alloc(nc, pools)
                    emit_phase_a_k(nc, pools, Kd, b + 1, identB, handles[b + 1], 0, bg)
                elif qi == 1:
                    emit_phase_a_k(nc, pools, Kd, b + 1, identB, handles[b + 1], 2, bg)
                    emit_phase_a_q(nc, pools, Qd, b + 1, identB, handles[b + 1], 0, bg)
                elif qi == 2:
                    emit_phase_a_v(nc, pools, Vd, b + 1, handles[b + 1])
                    emit_phase_a_k(nc, pools, Kd, b + 1, identB, handles[b + 1], 1, bg)
                elif qi == 3:
                    emit_phase_a_k(nc, pools, Kd, b + 1, identB, handles[b + 1], 3, bg)
                    emit_phase_a_q(nc, pools, Qd, b + 1, identB, handles[b + 1], 1, bg)
                elif qi == 4:
                    emit_phase_a_q(nc, pools, Qd, b + 1, identB, handles[b + 1], 2, bg)
                elif qi == 5:
                    emit_phase_a_q(nc, pools, Qd, b + 1, identB, handles[b + 1], 3, bg)

            qt2, ktp, vq4 = handles[b]
            emit_phase_b(
                nc, pools, Od, b, qt2, ktp, vq4, ones, wsel, ident2, prefetch, bg
            )
            while bg:
                bg.pop(0)()


_nc_cache = None


def build_nc():
    global _nc_cache
    if _nc_cache is not None:
        return _nc_cache
    nc = bacc.Bacc(None, target_bir_lowering=False)
    Qd = nc.declare_dram_parameter("Q", [BS, S, D], F32, isOutput=False)
    Kd = nc.declare_dram_parameter("K", [BS, S, D], F32, isOutput=False)
    Vd = nc.declare_dram_parameter("V", [BS, S, D], F32, isOutput=False)
    Od = nc.declare_dram_parameter("out", [BS, S, D], F32, isOutput=True)
    with TileContext(nc) as tc:
        build_body(nc, tc, Qd, Kd, Vd, Od)
    nc.finalize()
    _nc_cache = nc
    return nc


def kernel(Q, K, V):
    Q = np.asarray(Q, dtype=np.float32)
    K = np.asarray(K, dtype=np.float32)
    V = np.asarray(V, dtype=np.float32)
    nc = build_nc()
    in_maps = [
        {
            "Q": np.ascontiguousarray(Q[i * BS : (i + 1) * BS]),
            "K": np.ascontiguousarray(K[i * BS : (i + 1) * BS]),
            "V": np.ascontiguousarray(V[i * BS : (i + 1) * BS]),
        }
        for i in range(N_CORES)
    ]
    res = run_bass_kernel_spmd(nc, in_maps, core_ids=list(range(N_CORES)))
    return np.concatenate([res.results[i]["out"] for i in range(N_CORES)], axis=0)
